# revision 1
# baseline (speedup 1.0000x reference)
"""Trainium2 Bass kernel for nn_Blast: out = x @ (W0 + 1 bias^T) + bias
where W0 block (i_in, i_out) = Vt[i] @ diag(S[o,i]) @ U[o].

Factorized algorithm (per core, 256 tokens):
  midT[(o,r), tok] = sum_in A[in, (o,r)] * xT[in, tok]     (A = Vt*S, built on device)
  out[tok, oq]     = sum_r midT[(o,r), tok] * U''[o, r, q]

Layout: the 272 mid rows (16 o-blocks x 17) live at 32-aligned slots
(o -> psum group g=o//4, slot j=o%4, rows 32j..32j+16); A is zero-padded to
512 columns so the A-phase runs full-128-row matmuls (f32r forbids PE
subarray tiling, and only full-K matmul streams engage the PE's 2.4 GHz
activity monitor).

Bias trick: out = x@W0 + (rowsum(x)+1)*bias.  A has a 17th all-ones column
per o-block (-> rowsum in mid row 32j+16); each mid bank is opened by a
matmul writing 1.0 everywhere, so rank rows carry mid+1 and padding rows
carry 1.0; U'' row 16 = bias (multiplies rowsum+1), row 17 = -sum_r U[o,r]
(cancels the +1 pollution via the 1.0 padding row). U'' is zero-padded to
K=128 so the B-phase matmuls also run full-K (stay warm) and share one
weight load per group of four output blocks.

PE warmup: ~40 dummy full-K matmuls run during the input-DMA window; the
hardware activity monitor only unthrottles 1.2->2.4 GHz after ~a window of
contiguous full-K matmul activity, and low-K matmuls do not count.

Sharding: pure data-parallel over the 2048 tokens (8 cores x 256); the
small factors are replicated. x is fed pre-transposed (xT) from the host.
"""

import numpy as np

IN_DIM = 4096
OUT_DIM = 4096
BLOCK = 256
RANK = 16
B_IN = 16
B_OUT = 16
N_CORES = 8
TOK = 2048
TPC = TOK // N_CORES          # 256 tokens per core
RA = RANK + 1                 # 17: rank cols + rowsum col per o-block
KU = RANK + 2                 # 18: used rows of U'' per o-block
CP = 32                       # padded per-o column stride (32-aligned slots)
CAP = B_OUT * CP              # 512 padded columns of A
NCHUNK = IN_DIM // 128        # 32 K-chunks
NWARM = 28                    # PE warmup matmuls

_CACHE = {}

# test.py toggles; harness never touches these
TRACE = False
TRACE_DIR = None
LAST_RESULTS = None


def build_program():
    import concourse.mybir as mybir
    from concourse import bacc
    from concourse.tile import TileContext

    f32 = mybir.dt.float32
    f32r = mybir.dt.float32r

    nc = bacc.Bacc(trn_type="TRN2")
    xt_d = nc.dram_tensor("xt", (IN_DIM, TPC), f32r, kind="ExternalInput")
    vt_d = nc.dram_tensor("vt", (B_IN, BLOCK, CP), f32, kind="ExternalInput")
    s_d = nc.dram_tensor("s_flat", (1, B_IN * CAP), f32r, kind="ExternalInput")
    aship_d = nc.dram_tensor("aship", (B_IN // 2, 2 * 128, CAP), f32r, kind="ExternalInput")
    u_d = nc.dram_tensor("u_mat", (B_OUT, KU, BLOCK), f32r, kind="ExternalInput")
    w_d = nc.dram_tensor("wseed", (128, BLOCK), f32r, kind="ExternalInput")
    konst_d = nc.dram_tensor("konst", (1, 2 * TPC), f32r, kind="ExternalInput")
    out_d = nc.dram_tensor("out", (TPC, OUT_DIM), f32, kind="ExternalOutput")

    with TileContext(nc) as tc:
        from contextlib import ExitStack

        with ExitStack() as ctx:
            consts = ctx.enter_context(tc.tile_pool(name="consts", bufs=1))
            spool = ctx.enter_context(tc.tile_pool(name="spool", bufs=4))
            xpool = ctx.enter_context(tc.tile_pool(name="xpool", bufs=1))
            apool = ctx.enter_context(tc.tile_pool(name="apool", bufs=1))
            midsb = ctx.enter_context(tc.tile_pool(name="midsb", bufs=1))
            outsb = ctx.enter_context(tc.tile_pool(name="outsb", bufs=6))
            ps_mid = ctx.enter_context(
                tc.tile_pool(name="ps_mid", bufs=1, space="PSUM")
            )

            # ---- input loads ----
            # warm-up seed: first transfer on the sync queue
            wsb = consts.tile([128, BLOCK], f32r, name="wsb", tag="wsb")
            nc.sync.dma_start(out=wsb[:], in_=w_d[:])

            # memset can't produce f32r (ISA), so ones come via DMA:
            # konst = [ones(256) | zeros(256)]
            konst_sb = consts.tile([1, 2 * TPC], f32r, name="konst_sb", tag="konst_sb")
            nc.gpsimd.dma_start(out=konst_sb[:], in_=konst_d[:])
            ones_sb = konst_sb[0:1, 0:128]
            onestpc_sb = konst_sb[0:1, 0:TPC]

            s_sb = consts.tile([1, B_IN * CAP], f32r, name="s_sb", tag="s_sb")
            nc.gpsimd.dma_start(out=s_sb[:], in_=s_d[:])

            # all Vt chunks in one DMA: vt_all[p, i, h, r], h = 128-row half
            vt_all = consts.tile([128, B_IN * 2 * CP], f32, name="vt_all", tag="vt_all")
            nc.gpsimd.dma_start(
                out=vt_all[:].rearrange("p (i a r) -> p i a r", i=B_IN, a=2),
                in_=vt_d[:].rearrange("i (a p) r -> p i a r", p=128),
            )
            vt_v = vt_all[:].rearrange("p (i a r) -> p i a r", i=B_IN, a=2)

            # U'': usb[32*(o%4)+r, o*256+q] = U''[o,r,q]; one DMA per slot j
            usb = consts.tile([128, B_OUT * BLOCK], f32r, name="usb", tag="usb")
            for j in range(4):
                nc.gpsimd.dma_start(
                    out=usb[32 * j : 32 * j + KU, :]
                    .rearrange("r (g q) -> r g q", g=4)[:, :, j * BLOCK : (j + 1) * BLOCK],
                    in_=u_d[:].rearrange("(g jj) r q -> jj r g q", jj=4)[j],
                )

            # x^T chunk batches interleaved with shipped A chunks (even i)
            # on the sync queue; chunks for odd i are built on device below
            XGRP = 4
            xbatches = []
            ashipped = {}
            for b in range(NCHUNK // XGRP):
                xb = xpool.tile([128, XGRP * TPC], f32r, name=f"xb{b}", tag=f"xb{b}")
                nc.sync.dma_start(
                    out=xb[:].rearrange("p (k t) -> p k t", k=XGRP),
                    in_=xt_d[b * XGRP * 128 : (b + 1) * XGRP * 128, :].rearrange(
                        "(k p) t -> p k t", p=128
                    ),
                )
                xbatches.append(xb)
                i = 2 * b  # even i whose chunk pair ships whole
                if i < B_IN:
                    ab = apool.tile(
                        [128, 2 * CAP], f32r, name=f"ab{i}", tag=f"ab{i}"
                    )
                    # early pairs ride the sync ring between x batches; late
                    # pairs go via the GpSimd queue so the x tail isn't
                    # serialized behind them
                    eng = nc.sync if i <= 4 else nc.gpsimd
                    eng.dma_start(
                        out=ab[:].rearrange("p (two c) -> p two c", two=2),
                        in_=aship_d[i // 2].rearrange("(two p) c -> p two c", p=128),
                    )
                    ashipped[2 * i] = ab[:, 0:CAP]
                    ashipped[2 * i + 1] = ab[:, CAP : 2 * CAP]

            def xchunk(k):
                return xbatches[k // XGRP][:, (k % XGRP) * TPC : (k % XGRP + 1) * TPC]


            # ---- A-builds: S row broadcast (PE), stage (ACT), Vt*S (DVE/GPS)
            # These engines start as soon as s/vt land, overlapping the PE
            # warmup below; the A-phase then never waits on a build.
            midp = []
            abuilt = {}
            with tc.tile_pool(name="ps_pre", bufs=1, space="PSUM") as ps_pre:
                # ---- PE warmup while inputs stream in ----
                warm = ps_pre.tile([128, BLOCK], f32, name="warm", tag="warm", bufs=1)
                for w in range(NWARM):
                    nc.tensor.matmul(
                        warm[:],
                        lhsT=wsb[:, 0:128],
                        rhs=wsb[:],
                        start=True,
                        stop=True,
                        tile_position=(0, 0),
                    )

                for i in range(1, B_IN, 2):
                    sp = ps_pre.tile([128, CAP], f32, name="sp", tag="sp", bufs=3)
                    nc.tensor.matmul(
                        sp[:],
                        lhsT=ones_sb,
                        rhs=s_sb[0:1, i * CAP : (i + 1) * CAP],
                        start=True,
                        stop=True,
                        tile_position=(0, 0),
                    )
                    sps = spool.tile([128, CAP], f32, name="sps", tag="sps")
                    nc.scalar.copy(sps[:], sp[:])
                    for h in range(2):
                        k = 2 * i + h
                        a_t = apool.tile([128, CAP], f32r, name=f"a{k}", tag=f"a{k}")
                        eng = nc.gpsimd if i >= 13 else nc.vector
                        eng.tensor_mul(
                            a_t[:].rearrange("p (o r) -> p o r", r=CP),
                            vt_v[:, i, h, :]
                            .unsqueeze(1)
                            .broadcast_to([128, B_OUT, CP]),
                            sps[:].rearrange("p (o r) -> p o r", r=CP),
                        )
                        abuilt[k] = a_t

                # ---- open the mid banks with 1.0 everywhere ----
                for g in range(4):
                    mp = ps_mid.tile([128, TPC], f32, name=f"midp{g}", tag=f"midp{g}")
                    nc.tensor.matmul(
                        mp[:],
                        lhsT=ones_sb,
                        rhs=onestpc_sb,
                        start=True,
                        stop=False,
                        tile_position=(0, 0),
                    )
                    midp.append(mp)

                asbs = [
                    ashipped[k] if k in ashipped else abuilt[k]
                    for k in range(NCHUNK)
                ]

                # ---- phase A: midT accumulation over 32 K-chunks ----
                # a dummy warm matmul after every other chunk keeps the PE
                # activity monitor latched through DMA-starvation gaps
                for k in range(NCHUNK):
                    for g in range(4):
                        nc.tensor.matmul(
                            midp[g][:],
                            lhsT=asbs[k][:, g * 128 : (g + 1) * 128],
                            rhs=xchunk(k),
                            start=False,
                            stop=(k == NCHUNK - 1),
                            tile_position=(0, 0),
                        )
                    nfill = 2 if 6 <= k <= 24 else (1 if 2 <= k <= 26 else 0)
                    for _ in range(nfill):
                        nc.tensor.matmul(
                            warm[:],
                            lhsT=wsb[:, 0:128],
                            rhs=wsb[:],
                            start=True,
                            stop=True,
                            tile_position=(0, 0),
                        )

            # ---- midT to SBUF, one token-half at a time so phase B can
            # start on half 0 while half 1 still copies ----
            mids = []
            for g in range(4):
                ms = midsb.tile([128, TPC], f32r, name=f"mids{g}", tag=f"mids{g}")
                mids.append(ms)
            for tt in range(2):
                for g in range(4):
                    sl = (slice(None), slice(tt * 128, (tt + 1) * 128))
                    if (g + tt) % 2 == 0:
                        nc.scalar.copy(mids[g][sl], midp[g][sl])
                    else:
                        nc.vector.tensor_copy(mids[g][sl], midp[g][sl])

            # ---- phase B: out tiles [128 tok, 256 q], K=128 ----
            ps_out = ctx.enter_context(
                tc.tile_pool(name="ps_out", bufs=4, space="PSUM")
            )
            OGRP = 4  # o-blocks per output DMA; o//4 == g inside a group
            for tt in range(TPC // 128):
                for og in range(B_OUT // OGRP):
                    osb_t = outsb.tile(
                        [128, OGRP * BLOCK], f32, name="osb", tag="osb"
                    )
                    for oo in range(OGRP):
                        o = og * OGRP + oo
                        po = ps_out.tile([128, BLOCK], f32, name="po", tag="po")
                        j = o % 4
                        nc.tensor.matmul(
                            po[:],
                            lhsT=mids[o // 4][
                                32 * j : 32 * j + KU, tt * 128 : (tt + 1) * 128
                            ],
                            rhs=usb[
                                32 * j : 32 * j + KU, o * BLOCK : (o + 1) * BLOCK
                            ],
                            start=True,
                            stop=True,
                            tile_position=(32 * j, 0),
                        )
                        if o % 2 == 0:
                            nc.vector.tensor_copy(
                                osb_t[:, oo * BLOCK : (oo + 1) * BLOCK], po[:]
                            )
                        else:
                            nc.scalar.copy(
                                osb_t[:, oo * BLOCK : (oo + 1) * BLOCK], po[:]
                            )
                    nc.sync.dma_start(
                        out=out_d[
                            tt * 128 : (tt + 1) * 128,
                            og * OGRP * BLOCK : (og + 1) * OGRP * BLOCK,
                        ],
                        in_=osb_t[:],
                    )

    nc.compile()
    return nc


def prep_inputs(x, S, U, Vt, bias):
    """Host-side layout prep. Returns per-core input maps."""
    x = np.ascontiguousarray(np.asarray(x, dtype=np.float32))
    S = np.asarray(S, dtype=np.float32)
    U = np.asarray(U, dtype=np.float32)
    Vt = np.asarray(Vt, dtype=np.float32)
    bias = np.asarray(bias, dtype=np.float32)

    xt = np.ascontiguousarray(x.reshape(TOK, IN_DIM).T)  # (4096, 2048)

    vt_aug = np.zeros((B_IN, BLOCK, CP), np.float32)
    vt_aug[:, :, :RANK] = Vt
    vt_aug[:, :, RANK] = 1.0  # rowsum column

    # s_flat[0, i*CAP + o*CP + r] = S_aug[o, i, r]; pad r>=17 stays 0
    s_pad = np.zeros((B_IN, B_OUT, CP), np.float32)
    s_pad[:, :, :RANK] = S.transpose(1, 0, 2)
    s_pad[:, :, RANK] = 1.0  # rowsum column weight
    s_flat = np.ascontiguousarray(s_pad.reshape(1, B_IN * CAP))

    # row 16 multiplies mid row (rowsum+1) -> bias;  row 17 multiplies the
    # constant 1.0 padding row and cancels the +1 bank-init pollution of the
    # 16 rank rows: -sum_r U[o,r,:]
    bias_row = bias.reshape(B_OUT, 1, BLOCK)
    comp_row = -U.sum(axis=1, keepdims=True)  # (16, 1, 256)
    u_aug = np.ascontiguousarray(
        np.concatenate([U, bias_row, comp_row], axis=1)
    )  # (16, 18, 256)

    # shipped A chunk pairs (even i): A[(i,p),(o,r)] = vt_aug[i,p,r]*s_pad[i,o,r]
    a_even = np.einsum(
        "ipr,ior->ipor", vt_aug[0::2], s_pad[0::2]
    )  # (8, 256, 16, 32)
    aship = np.ascontiguousarray(a_even.reshape(B_IN // 2, 2 * 128, CAP))

    rng = np.random.default_rng(0)
    wseed = rng.standard_normal((128, BLOCK), dtype=np.float32)

    konst = np.zeros((1, 2 * TPC), np.float32)
    konst[0, :TPC] = 1.0

    in_maps = []
    for c in range(N_CORES):
        in_maps.append(
            {
                "xt": np.ascontiguousarray(xt[:, c * TPC : (c + 1) * TPC]),
                "vt": vt_aug,
                "s_flat": s_flat,
                "aship": aship,
                "u_mat": u_aug,
                "wseed": wseed,
                "konst": konst,
            }
        )
    return in_maps


def kernel(x, S, U, Vt, bias):
    global LAST_RESULTS
    from concourse.bass_utils import run_bass_kernel_spmd

    if "nc" not in _CACHE:
        _CACHE["nc"] = build_program()
    nc = _CACHE["nc"]

    in_maps = prep_inputs(x, S, U, Vt, bias)
    res = run_bass_kernel_spmd(
        nc, in_maps, list(range(N_CORES)), trace=TRACE, tmpdir=TRACE_DIR
    )
    LAST_RESULTS = res
    out = np.concatenate([res.results[c]["out"] for c in range(N_CORES)], axis=0)
    return out.reshape(2, TOK // 2, OUT_DIM)



# revision 4
# speedup vs baseline: 1.5740x; 1.5740x over previous
"""Trainium2 Bass kernel for nn_Blast: out = x @ (W0 + 1 bias^T) + bias
where W0 block (i_in, i_out) = Vt[i] @ diag(S[o,i]) @ U[o].

z-factorized algorithm (per core, 256 tokens, all-bf16 streams):
  z[(i,r), t]   = sum_p Vt'[i,p,r] * xT[i-block p, t]   (32 MMs, M=17)
  mid[(o,r), t] = sum_(i,r') W2[(i,r'),(o,r)] * z       (16 MMs, W2 = S
                                                         scattered, host-built)
  out[t, oq]    = sum_r mid[(o,r), t] * U''[o,r,q]      (K=18 row-strip MMs,
                                                         4 concurrent)

Layouts: z lives in 4 PSUM groups x 4 col-strip slots (i -> group i//4,
rows 32*(i%4)..+17); mid in 4 PSUM groups x 4 slots (o -> group o//4, rows
32*(o%4)..+18).  Vt' has a 17th all-ones column so z row 32m+16 is the
block-colsum of x; W2 routes those to every mid rowsum row (32j+16).

Bias trick: out = x@W0 + (rowsum(x)+1)*bias.  Each z bank is opened by a
K=1 matmul writing 1/16 everywhere, so the 16 colsum rows sum to
rowsum+1 in the mid rowsum row; z rows 32m+17 stay at exactly 1/16 and
W2 (value 16 at one of them) turns that into a constant-1 mid row
(32j+17).  U'' row 16 = bias (x rowsum+1), row 17 cancels the 1/16
pollution of the rank rows: -(1/16)*sum_r (sum_i S[o,i,r]) U[o,r,:].

Everything streamed in bf16 (PSUM accumulates f32); host pre-transposes
x, pre-builds W2/Vt'/U'', and upcasts the bf16 output back to f32.
Sharding: pure data-parallel over the 2048 tokens (8 cores x 256).
"""

import numpy as np

IN_DIM = 4096
OUT_DIM = 4096
BLOCK = 256
RANK = 16
B_IN = 16
B_OUT = 16
N_CORES = 8
TOK = 2048
TPC = TOK // N_CORES          # 256 tokens per core
RA = RANK + 1                 # 17: rank cols + colsum col per chunk
KU = RANK + 2                 # 18: used rows of U'' / mid per o-block
NCHUNK = IN_DIM // 128        # 32 K-chunks
NWARM = 6                     # PE warmup matmuls
CINIT = 1.0 / 16.0            # z bank init constant

_CACHE = {}

# test.py toggles; harness never touches these
TRACE = False
TRACE_DIR = None
LAST_RESULTS = None


def build_program():
    import concourse.mybir as mybir
    from concourse import bacc
    from concourse.tile import TileContext

    f32 = mybir.dt.float32
    bf16 = mybir.dt.bfloat16

    nc = bacc.Bacc(trn_type="TRN2")
    xt_d = nc.dram_tensor("xt", (IN_DIM, TPC), bf16, kind="ExternalInput")
    vt_d = nc.dram_tensor("vt", (128, NCHUNK * RA), bf16, kind="ExternalInput")
    w2_d = nc.dram_tensor("w2", (128, 4 * 4 * 128), bf16, kind="ExternalInput")
    u_d = nc.dram_tensor("u_mat", (B_OUT, KU, BLOCK), bf16, kind="ExternalInput")
    w_d = nc.dram_tensor("wseed", (128, BLOCK), bf16, kind="ExternalInput")
    konst_d = nc.dram_tensor("konst", (1, 128 + BLOCK), bf16, kind="ExternalInput")
    out_d = nc.dram_tensor("out", (TPC, OUT_DIM), bf16, kind="ExternalOutput")

    with TileContext(nc) as tc:
        from contextlib import ExitStack

        with ExitStack() as ctx:
            consts = ctx.enter_context(tc.tile_pool(name="consts", bufs=1))
            xpool = ctx.enter_context(tc.tile_pool(name="xpool", bufs=1))
            zsb = ctx.enter_context(tc.tile_pool(name="zsb", bufs=1))
            midsb = ctx.enter_context(tc.tile_pool(name="midsb", bufs=1))
            outsb = ctx.enter_context(tc.tile_pool(name="outsb", bufs=6))
            ps_mid = ctx.enter_context(
                tc.tile_pool(name="ps_mid", bufs=1, space="PSUM")
            )

            # ---- input loads ----
            # warm-up seed + constants ride the sync queue first
            wsb = consts.tile([128, BLOCK], bf16, name="wsb", tag="wsb")
            nc.sync.dma_start(out=wsb[:], in_=w_d[:])
            konst_sb = consts.tile(
                [1, 128 + BLOCK], bf16, name="konst_sb", tag="konst_sb"
            )
            nc.sync.dma_start(out=konst_sb[:], in_=konst_d[:])
            ones_sb = konst_sb[0:1, 0:128]
            crow_sb = konst_sb[0:1, 128 : 128 + BLOCK]

            # small factors on the gpsimd queue, in order of first use
            vt_sb = consts.tile([128, NCHUNK * RA], bf16, name="vt_sb", tag="vt_sb")
            nc.gpsimd.dma_start(out=vt_sb[:], in_=vt_d[:])
            w2_sb = consts.tile([128, 4 * 512], bf16, name="w2_sb", tag="w2_sb")
            nc.gpsimd.dma_start(out=w2_sb[:], in_=w2_d[:])

            # U'': usb[32*(o%4)+r, o*256+q] = U''[o,r,q]; one DMA per slot j
            usb = consts.tile([128, B_OUT * BLOCK], bf16, name="usb", tag="usb")
            for j in range(4):
                nc.gpsimd.dma_start(
                    out=usb[32 * j : 32 * j + KU, :]
                    .rearrange("r (g q) -> r g q", g=4)[:, :, j * BLOCK : (j + 1) * BLOCK],
                    in_=u_d[:].rearrange("(g jj) r q -> jj r g q", jj=4)[j],
                )

            # x^T chunk batches on the sync queue
            xbatches = []
            xslices = []
            XGRPS = [4, 4, 8, 8, 8]
            base = 0
            for b, xg in enumerate(XGRPS):
                xb = xpool.tile([128, xg * TPC], bf16, name=f"xb{b}", tag=f"xb{b}")
                nc.sync.dma_start(
                    out=xb[:].rearrange("p (k t) -> p k t", k=xg),
                    in_=xt_d[base * 128 : (base + xg) * 128, :].rearrange(
                        "(k p) t -> p k t", p=128
                    ),
                )
                for kk in range(xg):
                    xslices.append(xb[:, kk * TPC : (kk + 1) * TPC])
                xbatches.append(xb)
                base += xg

            # ---- PSUM z pool (+ warmup) ----
            mids_shuf = midsb.tile(
                [128, 4 * TPC], bf16, name="mids_shuf", tag="mids_shuf"
            )
            zts = []
            with tc.tile_pool(name="ps_z", bufs=1, space="PSUM") as ps_z:
                for g in range(4):
                    zt = ps_z.tile([128, TPC], f32, name=f"zp{g}", tag=f"zp{g}")
                    zts.append(zt)

                # warmups share z bank 0 (the init matmul clears it after)
                for w in range(NWARM):
                    nc.tensor.matmul(
                        zts[0][:],
                        lhsT=wsb[:, 0:128],
                        rhs=wsb[:],
                        start=True,
                        stop=True,
                        tile_position=(0, 0),
                    )

                # open z banks with CINIT everywhere (K=1 matmul)
                for g in range(4):
                    nc.tensor.matmul(
                        zts[g][:],
                        lhsT=ones_sb,
                        rhs=crow_sb,
                        start=True,
                        stop=False,
                        tile_position=(0, 0),
                    )

                # ---- phase Z: z[(i,r),t] accumulation, 2 chunks per i ----
                zcopies = []
                mixmm = []
                for i in range(B_IN):
                    g, mp = i // 4, i % 4
                    for h in range(2):
                        c = 2 * i + h
                        nc.tensor.matmul(
                            zts[g][32 * mp : 32 * mp + RA, :],
                            lhsT=vt_sb[:, RA * c : RA * (c + 1)],
                            rhs=xslices[c],
                            start=False,
                            stop=(mp == 3 and h == 1),
                            tile_position=(0, 32 * mp),
                            skip_group_check=True,
                        )
                    if mp == 3:
                        # group g complete: stage to SBUF (bf16) and mix
                        zc = zsb.tile([128, TPC], bf16, name=f"zsb{g}", tag=f"zsb{g}")
                        if g % 2 == 0:
                            nc.vector.tensor_copy(zc[:], zts[g][:])
                        else:
                            nc.scalar.copy(zc[:], zts[g][:])
                        zcopies.append(zc)

                # ---- mix: mid[(o,r),t] = W2^T z, into 4 slot-layout banks ----
                midp = []
                for t in range(4):
                    mp_t = ps_mid.tile(
                        [128, TPC], f32, name=f"midp{t}", tag=f"midp{t}"
                    )
                    midp.append(mp_t)
                for g in range(4):
                    for t in range(4):
                        nc.tensor.matmul(
                            midp[t][:],
                            lhsT=w2_sb[:, 512 * g + 128 * t : 512 * g + 128 * (t + 1)],
                            rhs=zcopies[g][:],
                            start=(g == 0),
                            stop=(g == 3),
                            tile_position=(0, 0),
                        )

            # ---- mid to SBUF (bf16), full tiles, partition-preserving ----
            for t in range(4):
                dst = mids_shuf[:, t * TPC : (t + 1) * TPC]
                if t % 2 == 0:
                    nc.vector.tensor_copy(dst, midp[t][:])
                else:
                    nc.scalar.copy(dst, midp[t][:])

            # ---- phase B: out tiles [128 tok, 256 q], K=18, 4 row-strips ----
            ps_out = ctx.enter_context(
                tc.tile_pool(name="ps_out", bufs=4, space="PSUM")
            )
            nout = 0
            for tt in range(TPC // 128):
                for t in range(4):
                    osb_t = outsb.tile(
                        [128, 4 * BLOCK], bf16, name="osb", tag="osb"
                    )
                    for j in range(4):
                        o = 4 * t + j
                        po = ps_out.tile([128, BLOCK], f32, name="po", tag="po")
                        nc.tensor.matmul(
                            po[:],
                            lhsT=mids_shuf[
                                32 * j : 32 * j + KU,
                                t * TPC + tt * 128 : t * TPC + (tt + 1) * 128,
                            ],
                            rhs=usb[
                                32 * j : 32 * j + KU, o * BLOCK : (o + 1) * BLOCK
                            ],
                            start=True,
                            stop=True,
                            tile_position=(32 * j, 0),
                        )
                        eng = [nc.vector.tensor_copy, nc.scalar.copy][nout % 2]
                        eng(osb_t[:, j * BLOCK : (j + 1) * BLOCK], po[:])
                        nout += 1
                    nc.scalar.dma_start(
                        out=out_d[
                            tt * 128 : (tt + 1) * 128,
                            t * 4 * BLOCK : (t + 1) * 4 * BLOCK,
                        ],
                        in_=osb_t[:],
                    )

    nc.compile()
    return nc


def prep_inputs(x, S, U, Vt, bias):
    """Host-side layout prep. Returns per-core input maps."""
    import ml_dtypes

    bf = ml_dtypes.bfloat16
    x = np.asarray(x, dtype=np.float32)
    S = np.asarray(S, dtype=np.float32)
    U = np.asarray(U, dtype=np.float32)
    Vt = np.asarray(Vt, dtype=np.float32)
    bias = np.asarray(bias, dtype=np.float32)

    xt = np.ascontiguousarray(x.reshape(TOK, IN_DIM).T.astype(bf))  # (4096, 2048)

    # vt_sb[p, 17c + r] = Vt[i, 128h+p, r] (c = 2i+h), col 16 = ones
    vt_aug = np.ones((B_IN, BLOCK, RA), np.float32)
    vt_aug[:, :, :RANK] = Vt
    vt_host = np.ascontiguousarray(
        vt_aug.reshape(B_IN * 2, 128, RA)  # (c, p, r)
        .transpose(1, 0, 2)                # (p, c, r)
        .reshape(128, NCHUNK * RA)
        .astype(bf)
    )

    # W2[(g=i//4, 32*(i%4)+r'), (t=o//4, 32*(o%4)+rr)] block layout:
    #   r'<16, rr=r':  S[o, i, r']
    #   r'=16, rr=16:  1            (colsum rows -> rowsum row)
    #   r'=17 (g=0,i%4=0 only), rr=17: 16   (CINIT row -> const-1 row)
    w2 = np.zeros((4, 128, 4, 128), np.float32)  # (g, zrow, t, midcol)
    for i in range(B_IN):
        g, mp = i // 4, i % 4
        for o in range(B_OUT):
            t, j = o // 4, o % 4
            for r in range(RANK):
                w2[g, 32 * mp + r, t, 32 * j + r] = S[o, i, r]
            w2[g, 32 * mp + RANK, t, 32 * j + RANK] = 1.0
    for o in range(B_OUT):
        t, j = o // 4, o % 4
        w2[0, RANK + 1, t, 32 * j + RANK + 1] = 16.0
    w2_host = np.ascontiguousarray(
        w2.transpose(1, 0, 2, 3).reshape(128, 4 * 512).astype(bf)
    )

    # U'' rows: [U (16); bias (1); comp (1)]
    bias_row = bias.reshape(B_OUT, 1, BLOCK)
    s_sum = S.sum(axis=1)  # (B_OUT, RANK): sum_i S[o,i,r]
    comp_row = -(CINIT) * np.einsum("or,orq->oq", s_sum, U)[:, None, :]
    u_aug = np.ascontiguousarray(
        np.concatenate([U, bias_row, comp_row], axis=1).astype(bf)
    )  # (16, 18, 256)

    rng = np.random.default_rng(0)
    wseed = rng.standard_normal((128, BLOCK), dtype=np.float32).astype(bf)

    konst = np.zeros((1, 128 + BLOCK), np.float32)
    konst[0, :128] = 1.0
    konst[0, 128:] = CINIT
    konst = konst.astype(bf)

    in_maps = []
    for c in range(N_CORES):
        in_maps.append(
            {
                "xt": np.ascontiguousarray(xt[:, c * TPC : (c + 1) * TPC]),
                "vt": vt_host,
                "w2": w2_host,
                "u_mat": u_aug,
                "wseed": wseed,
                "konst": konst,
            }
        )
    return in_maps


def kernel(x, S, U, Vt, bias):
    global LAST_RESULTS
    from concourse.bass_utils import run_bass_kernel_spmd

    if "nc" not in _CACHE:
        _CACHE["nc"] = build_program()
    nc = _CACHE["nc"]

    in_maps = prep_inputs(x, S, U, Vt, bias)
    res = run_bass_kernel_spmd(
        nc, in_maps, list(range(N_CORES)), trace=TRACE, tmpdir=TRACE_DIR
    )
    LAST_RESULTS = res
    out = np.concatenate(
        [res.results[c]["out"].astype(np.float32) for c in range(N_CORES)], axis=0
    )
    return out.reshape(2, TOK // 2, OUT_DIM)
